# revision 27
# baseline (speedup 1.0000x reference)
"""Trainium2 Bass kernel for nn_BaseIODEModel (GNN message-passing ODE field).

Data-parallel over trajectories: z [81920, 4] is split across 8 NeuronCores
along dim 0 (1024 trajectories / 10240 rows per core); the small MLP weights
are replicated. Edge gather/softplus/sum is local per trajectory, so there is
no cross-device communication.

Per-core program, fp16 matmul datapath (PE runs 2-byte dtypes at 1 col/cycle
vs 2-4 cycles for fp32r; fp32 PSUM accumulation throughout; ranges verified:
max |a_r + b_s| < 9.2 so exp products stay < 1e4, well inside fp16):

  zT = transpose(z)                          [4, 1280] per group (PE)
  interaction layer 0 factorizes over edges:
       pre(r,s) = a_r + b_s + ib0,  a = [iW0_p; iW0_vr].T z,
                                    b = [-iW0_p; iW0_vs].T z
       ea = exp(a + ib0), eb = exp(b)   (ACT, node columns only)
       t0(k,d,r,t) = ea_r * eb_{(r+d)%B}  (DVE fp16 2x, one op per shift d)
       g0 = ln(1 + t0[k])               (ACT, one 2880-col op per block)
  layer 1:  u = exp(iW1.T g0 + ib1)     (ACT, PSUM in, f32 out)
            h1e = ln(1 + u)             (ACT, one 2880-col op per block)
  hsum = sum_d h1e[:, d]                (DVE fp16 2x tree, 4 adds)
  dz = fW2.T h1s + iW2.T hsum + bias    (PE, 2 matmuls per block)
  self-dynamics MLP: same exp/ln softplus pairs on node columns.

ACT (the only transcendental engine; ~0.86 ns/col, dtype-independent) is the
roofline: 3 edge passes x 92160 cols + 6 node passes x 10240 cols per core.
All ACT instructions are 1280-2880 cols to amortize the ~200 ns access
latency, and the node-MLP/prefetch work is staged across the four edge
blocks of the previous group so ACT never waits on the PE or DVE.
"""

import numpy as np

import concourse.bass as bass
import concourse.hw_specs as _hw_specs
import concourse.mybir as _mybir_for_tables
from concourse import bacc


def _patch_activation_tables():
    """Make Exp and Ln resolve to the combined natural_log_exp_and_others
    ACT table set. Bacc's insert_act_table_loads picks the first set that
    contains each function, which puts Exp and Ln in two different sets and
    inserts a ~1.3us ACT_TABLE_LOAD at every exp<->ln alternation. Filtering
    the other sets' exp/ln entries keeps set ids stable (index into
    act_info.json) while forcing the shared set."""
    if getattr(_hw_specs, "_nle_patched", False):
        return
    orig = _hw_specs.get_activation_tables
    comb = "natural_log_exp_and_others"
    EXP = _mybir_for_tables.ActivationFunctionType.Exp
    LN = _mybir_for_tables.ActivationFunctionType.Ln

    def patched(module_arch):
        tables = orig(module_arch)
        if comb in tables and EXP in tables[comb] and LN in tables[comb]:
            for name, funcs in tables.items():
                if name != comb:
                    funcs.discard(EXP)
                    funcs.discard(LN)
        return tables

    _hw_specs.get_activation_tables = patched
    _hw_specs._nle_patched = True
    import concourse.bacc as _bacc_mod
    if getattr(_bacc_mod, "get_activation_tables", None) is orig:
        _bacc_mod.get_activation_tables = patched


_patch_activation_tables()
import concourse.mybir as mybir
import concourse.tile as tile
from concourse.bass_utils import run_bass_kernel_spmd
from concourse.masks import make_identity

F32 = mybir.dt.float32
F16 = mybir.dt.float16
AF = mybir.ActivationFunctionType

B = 10          # objects per trajectory
NDIM = 2
NF = 2 * NDIM   # 4 features per node
H = 128         # hidden width (both MLPs)

N_CORES = 8
N_TRAJ = 8192             # total trajectories
N_LOC = N_TRAJ // N_CORES  # 1024 trajectories per core
ROWS = N_LOC * B          # 10240 node rows per core
GT = 128                  # trajectories per group
NGROUP = N_LOC // GT      # 8 groups
GCOLS = GT * B            # 1280 node cols per group
TT = 32                   # trajectories per edge block
NBLK = GT // TT           # 4 edge blocks per group
GRID = TT * (B - 1) * B   # 2880 grid cols per block
HGT = GT // 2             # half-group store width (64 trajectories)

WEIGHT_NAMES = [
    "fW0", "fb0", "fW1", "fb1", "fW2", "fb2",
    "iW0", "ib0", "iW1", "ib1", "iW2", "ib2",
]


def build(ngroup=NGROUP):
    nc = bacc.Bacc()
    rows = ngroup * GCOLS

    z = nc.declare_dram_parameter("z", [NF, rows], F16, isOutput=False)
    w = {}
    for name, shp in [
        ("fW0", [NF, H]), ("fW1", [H, H]), ("fW2", [H, NF]),
        ("iW1", [H, H]), ("iW2", [H, NF]),
        ("Wa", [NF, H]), ("Wb", [NF, H]),
    ]:
        w[name] = nc.declare_dram_parameter(name, shp, F16, isOutput=False)
    for name, shp in [
        ("fb0", [H]), ("fb1", [H]), ("ib0", [H]), ("ib1", [H]),
        ("bias2", [NF]),
    ]:
        w[name] = nc.declare_dram_parameter(name, shp, F32, isOutput=False)
    out = nc.declare_dram_parameter("out", [NF, rows], F32, isOutput=True)

    # Feature-major DRAM layout, cols ordered (g, r, t) to match the
    # on-chip column order — layout conversion happens host-side for free.
    z_v = z.rearrange("f (g c) -> f g c", g=ngroup)
    out_v = out.rearrange("f (g r t) -> f g r t", g=ngroup, r=B)

    with tile.TileContext(nc) as tc:
        with (
            tc.tile_pool(name="const", bufs=1) as const,
            tc.tile_pool(name="zio", bufs=2) as zio,
            tc.tile_pool(name="nodes", bufs=2) as nodes,
            tc.tile_pool(name="grids", bufs=2) as grids,
            tc.tile_pool(name="outs", bufs=2) as outs,
            # One shared PSUM ring for all [128, x] matmul outputs:
            # 1536 f32 = 3 banks; x2 bufs = 6 banks.  dz: [4,320] x2.
            tc.tile_pool(name="ab_ps", bufs=2, space="PSUM") as ab_ps,
            tc.tile_pool(name="dz_ps", bufs=2, space="PSUM") as dz_ps,
        ):
            def ps_take():
                ab_take = ab_ps.tile([128, 1536], F32, tag="ab")
                return ab_take

            # weights arrive fp16 from host prep; DMA on the idle GpSimd
            # queue so the z-load path is not serialized behind them.
            def weight_tile(p, fdim, name):
                t = const.tile([p, fdim], F16, tag=f"w_{name}")
                nc.gpsimd.dma_start(out=t[:], in_=w[name][:])
                return t

            Wa_sb = weight_tile(NF, H, "Wa")
            Wb_sb = weight_tile(NF, H, "Wb")
            fW0_sb = weight_tile(NF, H, "fW0")
            fW1_sb = weight_tile(H, H, "fW1")
            fW2_sb = weight_tile(H, NF, "fW2")
            iW1_sb = weight_tile(H, H, "iW1")
            iW2_sb = weight_tile(H, NF, "iW2")

            def bias_col(p, name):
                t = const.tile([p, 1], F32, tag=f"bias_{name}")
                nc.gpsimd.dma_start(
                    out=t[:], in_=w[name].rearrange("(a b) -> a b", b=1))
                return t

            ib0_c = bias_col(H, "ib0")
            ib1_c = bias_col(H, "ib1")
            fb0_c = bias_col(H, "fb0")
            fb1_c = bias_col(H, "fb1")
            # bias2 = fb2 + 9*ib2 prepared host-side (param "bias2")
            bias2_c = bias_col(NF, "bias2")

            def z_load(g):
                # z arrives feature-major fp16 from host prep: direct DMA
                zT = nodes.tile([NF, GCOLS], F16)
                nc.sync.dma_start(out=zT[:], in_=z_v[:, g])
                return zT

            def mm_node(w_sb, rhs, act_func, act_bias, out_ap):
                # W.T @ rhs over GCOLS in bank-aligned chunks, then one
                # whole-width ACT pass PSUM -> SBUF.
                ps = ps_take()
                for c0, c1 in [(0, 512), (512, 1024), (1024, GCOLS)]:
                    nc.tensor.matmul(ps[:, c0:c1], w_sb[:], rhs[:, c0:c1],
                                     start=True, stop=True)
                nc.scalar.activation(out=out_ap, in_=ps[:, 0:GCOLS],
                                     func=act_func, bias=act_bias, scale=1.0)

            def node_stage1(zT):
                # edge layer-0 node terms: ea = exp(a+ib0), eb = exp(b) (x2)
                ea = nodes.tile([H, B, GT], F16, tag="ea")
                eb_ext = nodes.tile([H, 2 * B, GT], F16, tag="eb")
                ea_f = ea[:].rearrange("p r t -> p (r t)")
                eb_f = eb_ext[:].rearrange("p r t -> p (r t)")
                mm_node(Wa_sb, zT, AF.Exp, ib0_c[:], ea_f[:, 0:GCOLS])
                mm_node(Wb_sb, zT, AF.Exp, 0.0, eb_f[:, 0:GCOLS])
                nc.vector.tensor_copy(eb_f[:, GCOLS:2 * GCOLS],
                                      eb_f[:, 0:GCOLS])
                return ea, eb_ext

            def node_stage2(zT, ea, eb_ext):
                # self MLP layer 0, then the grid combine for the next group
                u0 = nodes.tile([H, GCOLS], F16, tag="u0")
                mm_node(fW0_sb, zT, AF.Exp, fb0_c[:], u0[:])
                h0s = nodes.tile([H, GCOLS], F16, tag="h0s")
                nc.scalar.activation(out=h0s[:], in_=u0[:], func=AF.Ln,
                                     bias=1.0, scale=1.0)
                # t0(k,d,r,t) = ea_r * eb_{r+d}; block-major so each grid ln
                # reads a contiguous 2880 cols.
                t0g = grids.tile([H, NBLK, B - 1, B, TT], F16, tag="t0g")
                ea_b = ea[:].rearrange("p r (k t) -> p k r t", k=NBLK)
                for d in range(1, B):
                    eb_b = eb_ext[:, d:d + B, :].rearrange(
                        "p r (k t) -> p k r t", k=NBLK)
                    nc.vector.tensor_mul(t0g[:, :, d - 1, :, :], ea_b, eb_b)
                return h0s, t0g

            def node_stage3(h0s):
                # self MLP layer 1
                u1 = nodes.tile([H, GCOLS], F16, tag="u1")
                mm_node(fW1_sb, h0s, AF.Exp, fb1_c[:], u1[:])
                h1s = nodes.tile([H, B, GT], F16, tag="h1s")
                nc.scalar.activation(
                    out=h1s[:].rearrange("p r t -> p (r t)"), in_=u1[:],
                    func=AF.Ln, bias=1.0, scale=1.0)
                return h1s

            def store_half(g, out_sb, half):
                # feature-major store; host un-transposes for free
                base = half * HGT
                nc.sync.dma_start(
                    out=out_v[:, g, :, base:base + HGT],
                    in_=out_sb[:, :, base:base + HGT])

            def grid_ln(t0g, k):
                g0 = grids.tile([H, GRID], F16, tag="g0", bufs=3)
                nc.scalar.activation(
                    out=g0[:],
                    in_=t0g[:, k].rearrange("p d r t -> p (d r t)"),
                    func=AF.Ln, bias=1.0, scale=1.0)
                return g0

            # prologue: group 0's node work
            zT0 = z_load(0)
            ea0, eb0 = node_stage1(zT0)
            h0s0, t0g0 = node_stage2(zT0, ea0, eb0)
            h1s0 = node_stage3(h0s0)

            # flat software pipeline over all ngroup*NBLK edge blocks: the
            # grid-ln lookahead (2 blocks) and the staged node prefetch
            # both cross group boundaries, so ACT never drains at a seam.
            t0g_by_g = {0: t0g0}
            h1s_by_g = {0: h1s0}
            st = {}
            out_sb = None
            nblocks = ngroup * NBLK

            L1_SPLIT = [(0, 1536), (1536, GRID - 1536)]

            def l1_mms(g0):
                # layer-1 matmuls for one block into two PSUM takes
                takes = []
                for base, width in L1_SPLIT:
                    ps = ps_take()
                    for c0 in range(0, width, 512):
                        c1 = min(width, c0 + 512)
                        nc.tensor.matmul(
                            ps[:, c0:c1], iW1_sb[:],
                            g0[:, base + c0:base + c1],
                            start=True, stop=True)
                    takes.append(ps)
                return takes

            pend = [grid_ln(t0g0, 0), grid_ln(t0g0, 1)]
            l1_cur = l1_mms(pend[0])
            for j in range(nblocks):
                g, k = divmod(j, NBLK)
                t0g = t0g_by_g[g]
                h1s = h1s_by_g[g]
                if k == 0:
                    out_sb = outs.tile([NF, B, GT], F32, tag="out_sb")
                tsl = slice(k * TT, (k + 1) * TT)
                g0 = pend.pop(0)

                # the NEXT block's L1 matmuls go on the PE queue first; the
                # PSUM ring paces them behind this block's exp reads.
                l1_nxt = l1_mms(pend[0]) if pend else None

                # u = exp(iW1.T g0 + ib1) from this block's PSUM takes
                u_e = grids.tile([H, GRID], F16, tag="u_e")
                for (base, width), ps in zip(L1_SPLIT, l1_cur):
                    nc.scalar.activation(
                        out=u_e[:, base:base + width], in_=ps[:, 0:width],
                        func=AF.Exp, bias=ib1_c[:], scale=1.0)
                l1_cur = l1_nxt
                h1e = grids.tile([H, B - 1, B, TT], F16, tag="h1e")
                nc.scalar.activation(
                    out=h1e[:].rearrange("p d r t -> p (d r t)"),
                    in_=u_e[:], func=AF.Ln, bias=1.0, scale=1.0)

                # sum over the 9 senders on DVE (fp16 2x) so dz is two
                # matmuls instead of ten.
                s4 = grids.tile([H, 4, B, TT], F16, tag="s4")
                nc.vector.tensor_add(s4[:], h1e[:, 0:4, :, :],
                                     h1e[:, 4:8, :, :])
                s2 = grids.tile([H, 2, B, TT], F16, tag="s2")
                nc.vector.tensor_add(s2[:], s4[:, 0:2, :, :],
                                     s4[:, 2:4, :, :])
                s3 = grids.tile([H, B, TT], F16, tag="s3")
                nc.vector.tensor_add(s3[:], s2[:, 0, :, :], s2[:, 1, :, :])
                hsum = grids.tile([H, B, TT], F16, tag="hsum")
                nc.vector.tensor_add(hsum[:], s3[:], h1e[:, 8, :, :])

                # dz = fW2.T h1s + iW2.T hsum
                dzp = dz_ps.tile([NF, B * TT], F32)
                nc.tensor.matmul(dzp[:], fW2_sb[:],
                                 h1s[:, :, tsl], start=True, stop=False)
                nc.tensor.matmul(dzp[:], iW2_sb[:], hsum[:],
                                 start=False, stop=True)
                nc.vector.tensor_scalar_add(
                    out_sb[:, :, tsl], dzp[:], bias2_c[:])

                # staged prefetch of the next group's node work, one slice
                # per block, emitted AFTER the block's critical chain so it
                # fills ACT/PE idle instead of delaying it.
                if g + 1 < ngroup:
                    if k == 0:
                        st["zT"] = z_load(g + 1)
                    elif k == 1:
                        st["ea"], st["eb"] = node_stage1(st["zT"])
                    elif k == 2:
                        st["h0s"], t0g_by_g[g + 1] = node_stage2(
                            st["zT"], st["ea"], st["eb"])
                    elif k == 3:
                        h1s_by_g[g + 1] = node_stage3(st["h0s"])

                # queue the ln two blocks ahead (its t0g was produced by the
                # stage-2 prefetch emitted just above when crossing groups)
                if j + 2 < nblocks:
                    g2, k2 = divmod(j + 2, NBLK)
                    pend.append(grid_ln(t0g_by_g[g2], k2))

                if k == 2:
                    store_half(g, out_sb, 0)
                elif k == 3:
                    store_half(g, out_sb, 1)
                    t0g_by_g.pop(g, None)
                    h1s_by_g.pop(g, None)

    nc.finalize()
    return nc


_NC_CACHE = {}


def _get_nc():
    if "nc" not in _NC_CACHE:
        _NC_CACHE["nc"] = build()
    return _NC_CACHE["nc"]


def run(inputs, trace=False, **kwargs):
    """Shard, run on 8 cores, gather. Returns (out, BassKernelResults)."""
    nc = _get_nc()
    z = np.asarray(inputs["z"], dtype=np.float32)
    assert z.shape == (N_TRAJ * B, NF), z.shape
    # per-core feature-major fp16 with cols ordered (g, r, t)
    z5 = z.reshape(N_CORES, NGROUP, GT, B, NF).astype(np.float16)
    z_fm = np.ascontiguousarray(
        z5.transpose(0, 4, 1, 3, 2).reshape(N_CORES, NF, ROWS))
    w32 = {k: np.asarray(inputs[k], dtype=np.float32) for k in WEIGHT_NAMES}
    iW0 = w32["iW0"]
    weights = {}
    for k in ("fW0", "fW1", "fW2", "iW1", "iW2"):
        weights[k] = np.ascontiguousarray(w32[k].astype(np.float16))
    weights["Wa"] = np.ascontiguousarray(iW0[0:NF].astype(np.float16))
    weights["Wb"] = np.ascontiguousarray(np.concatenate(
        [-iW0[0:NDIM], iW0[2 * NDIM:3 * NDIM]], axis=0).astype(np.float16))
    for k in ("fb0", "fb1", "ib0", "ib1"):
        weights[k] = np.ascontiguousarray(w32[k])
    weights["bias2"] = np.ascontiguousarray(
        w32["fb2"] + (B - 1) * w32["ib2"])
    in_maps = []
    for c in range(N_CORES):
        m = dict(weights)
        m["z"] = z_fm[c]
        in_maps.append(m)
    res = run_bass_kernel_spmd(nc, in_maps, list(range(N_CORES)),
                               trace=trace, **kwargs)
    outs_fm = [res.results[c]["out"].reshape(NF, NGROUP, B, GT)
               for c in range(N_CORES)]
    out = np.concatenate(
        [o.transpose(1, 3, 2, 0).reshape(ROWS, NF) for o in outs_fm], axis=0)
    out = np.ascontiguousarray(out)
    return out, res


def kernel(**inputs) -> np.ndarray:
    out, _ = run(inputs)
    return out
